# revision 19
# baseline (speedup 1.0000x reference)
"""BeatPooling segment-mean kernel for 8 Trainium2 NeuronCores.

Reference computation (per batch row):
    s = clip(bounds[:, 0], 0, T-1); e = max(s+1, min(bounds[:, 1], T))
    mean[m] = sum(frame[s_m:e_m]) / (e_m - s_m)
    out = concat([mean, fourier(pos)], -1) @ W + b         # [M, D]

Sharding: data-parallel over B (one batch row per core).

Algorithm (per core). Every segment sum is cs[e-1] - cs[s-1] where cs is
the inclusive prefix sum over frames. Decompose cs[pos] into a block part
and a within-block part at 128-frame granularity:

    cs[pos] = C[K-1] + P[pos],  K = pos >> 7
    C[k]    = sum of block sums for blocks 0..k   (C[-1] = 0)
    P[pos]  = prefix of block K up to pos

The frame stream is consumed by ONE f32r matmul per 128-frame block:
the stationary operand holds up to 32 prefix-mask columns (one per
boundary position falling in that block, plus an all-ones column for the
block sum), the moving operand is the [128, 512] frame tile.  This
produces every P value and block sum the output needs while streaming at
1 cycle/row -- no big transpose, no per-element scan, no fp32->bf16
cast.  The mask columns are value-dependent but are *data*: an on-device
gpsimd gather picks them out of a static [128, 128] prefix-mask table
using host-computed indices, so the compiled program itself is
input-independent.

The small [slots, D] result (4 MiB vs the 16 MiB frame row) is PE-
transposed into a [d, slot] table, block sums are scanned into C, and a
single gpsimd gather fetches (P_s, C_s, P_e, C_e) per segment.  Two adds
and a subtract form the segment sums, which go through the [D, D]
projection in f32r; the fourier/bias term ff @ W[D:] + b is computed on
device from a tiny host-packed [34+34, 512] tensor, and the final
scalar_tensor_tensor fuses the 1/count scale and bias add.

All small constants travel in one packed int16 bundle: the baseline lost
~50 us draining dozens of tiny per-partition DMA descriptors after the
16 MiB stream.
"""

import math

import numpy as np

import concourse.bacc as bacc
import concourse.mybir as mybir
from concourse import bass_utils
from concourse.tile import TileContext

B, T, D, M = 8, 8192, 512, 512
POS_DIM = 32
P = 128
N_CORES = 8
NB = T // P            # 64 blocks of 128 frames
GROUPS = 8             # stream groups (8 blocks = 2 MiB each)
BPG = NB // GROUPS     # blocks per group
DC = D // P            # 4 d-chunks
MC = M // P            # 4 m-chunks

F32 = mybir.dt.float32
F32R = mybir.dt.float32r
I16 = mybir.dt.int16


def _layout(S):
    """Table + bundle layout for a given slot count per block."""
    nslot = NB * S
    cbase = nslot                  # elem-col where the C region starts
    zcol = cbase + NB              # elem-col of the zero column
    nelem = zcol + 8               # table elem-cols total (d=4 interleave)
    assert nelem * DC * 4 // 4 <= 2 ** 15, f"gather table too big for S={S}"
    # bundle layout (int16 cols): uidx | gidx | recip | U | ident
    uidx0 = 0
    gidx0 = uidx0 + nslot // 16
    recip0 = gidx0 + 4 * M // 16
    u0 = recip0 + 2 * MC
    ident0 = u0 + 2 * P
    cols = ident0 + 2 * P
    return dict(nslot=nslot, cbase=cbase, zcol=zcol, nelem=nelem,
                uidx0=uidx0, gidx0=gidx0, recip0=recip0, u0=u0,
                ident0=ident0, cols=cols)


_CACHED_NC = {}


def _build_nc(S):
    L = _layout(S)
    NSLOT = L["nslot"]
    CBASE, ZCOL, NELEM = L["cbase"], L["zcol"], L["nelem"]
    TPB = P // S           # blocks per P' psum tile (4 for S=32)

    nc = bacc.Bacc("TRN2", target_bir_lowering=False, debug=False,
                   num_devices=N_CORES)

    frame = nc.dram_tensor("frame", [T, D], F32R, kind="ExternalInput")
    w1p_in = nc.dram_tensor("w1p", [P, DC * D], F32R, kind="ExternalInput")
    ffw2_in = nc.dram_tensor("ffw2", [P, D], F32R, kind="ExternalInput")
    bund_in = nc.dram_tensor("bund", [P, L["cols"]], I16,
                             kind="ExternalInput")
    out = nc.dram_tensor("out", [M, D], F32, kind="ExternalOutput")

    add = mybir.AluOpType.add
    mult = mybir.AluOpType.mult
    bypass = mybir.AluOpType.bypass
    sub = mybir.AluOpType.subtract

    with TileContext(nc, num_cores=N_CORES) as tc:
        with (
            tc.tile_pool(name="const", bufs=1) as const,
            tc.tile_pool(name="staging", bufs=2) as staging,
            tc.tile_pool(name="pvac", bufs=2) as pvac,
            tc.tile_pool(name="psum", bufs=8, space="PSUM") as psum,
            tc.tile_pool(name="outp", bufs=2) as outp,
        ):
            # ---- long-lived tiles -------------------------------------
            bund = const.tile([P, L["cols"]], I16, name="bund")
            uslots = const.tile([P, NSLOT], F32R, name="uslots")
            w1t = const.tile([P, DC * D], F32R, name="w1t")
            ffa = const.tile([64, D], F32R, name="ffa")
            ffb = const.tile([64, D], F32R, name="ffb")
            table = const.tile([P, NELEM * DC], F32, name="table")
            gath = const.tile([P, 4 * M * DC], F32, name="gath")
            tmpe = const.tile([P, M * DC], F32, name="tmpe")
            tmps = const.tile([P, M * DC], F32, name="tmps")
            seg = const.tile([P, M * DC], F32R, name="seg")
            biassb = const.tile([P, MC * D], F32, name="biassb")

            uidx_v = bund[:, L["uidx0"]:L["gidx0"]]
            gidx_v = bund[:, L["gidx0"]:L["recip0"]]
            recip_v = bund[:, L["recip0"]:L["u0"]].bitcast(F32)
            u_v = bund[:, L["u0"]:L["ident0"]].bitcast(F32)
            ident_v = bund[:, L["ident0"]:L["cols"]].bitcast(F32)

            # ---- tiny constant DMAs (one per ring, front of queue) ----
            nc.sync.dma_start(bund[:], bund_in.ap())
            nc.scalar.dma_start(ffa[:], ffw2_in.ap()[0:64, :])
            nc.scalar.dma_start(ffb[:], ffw2_in.ap()[64:128, :])

            # zero column for s==0 / K==0 gathers
            nc.vector.memset(table[:, ZCOL * DC:NELEM * DC], 0.0)

            # boundary prefix-mask weights: uslots[:, k*S+j] = U[:, off_j(k)]
            # (gath doubles as scratch for the unrounded masks; the
            # main gather overwrites it much later)
            nc.gpsimd.ap_gather(
                gath[:, 0:NSLOT], u_v, uidx_v,
                channels=P, num_elems=P, d=1, num_idxs=NSLOT,
            )
            nc.vector.tensor_scalar_add(
                out=uslots[:], in0=gath[:, 0:NSLOT], scalar1=0.0)

            # fourier/bias term: bias[m, j] = ff[m] @ W2 + b  (34-row f32r
            # matmuls from the packed ffw2 tile), evacuated to SBUF early
            for mt in range(MC):
                bps = psum.tile([P, D], F32, name=f"bps_{mt}", tag="ps")
                nc.tensor.matmul(
                    bps[:],
                    lhsT=ffa[:, mt * P:(mt + 1) * P],
                    rhs=ffb[:],
                    start=True, stop=True,
                )
                nc.scalar.copy(biassb[:, mt * D:(mt + 1) * D], bps[:])

            # ---- stream frame: one edge-matmul per 128-frame block ----
            frame_g = frame.ap().rearrange("(g b p) d -> g p b d", p=P, b=BPG)
            pp = None
            for g in range(GROUPS):
                st = staging.tile([P, BPG * D], F32R, name="stage", tag="stage")
                ring = nc.sync if g % 2 == 0 else nc.scalar
                ring.dma_start(
                    st[:].rearrange("p (b d) -> p b d", b=BPG), frame_g[g])
                for b in range(BPG):
                    k = g * BPG + b
                    i = k % TPB
                    if i == 0:
                        pv = pvac.tile([P, D], F32, name="pv", tag="pv")
                    pp = psum.tile([S, D], F32, name=f"pp_{k}", tag="ps")
                    nc.tensor.matmul(
                        pp[:],
                        lhsT=uslots[:, k * S:(k + 1) * S],
                        rhs=st[:, b * D:(b + 1) * D],
                        start=True, stop=True,
                    )
                    nc.scalar.copy(pv[i * S:(i + 1) * S, :], pp[:])
                    if i == TPB - 1:
                        t = k // TPB
                        tp = psum.tile([P, D], F32, name=f"tp_{t}", tag="ps")
                        for c in range(DC):
                            nc.tensor.transpose(
                                tp[:, c * P:(c + 1) * P],
                                pv[:, c * P:(c + 1) * P],
                                ident_v,
                            )
                        # table f32col ((k*S+s)*4 + c) <- tp col (c*128+i*S+s)
                        nc.vector.tensor_scalar_add(
                            out=table[:, t * TPB * S * DC:(t + 1) * TPB * S * DC]
                                .rearrange("p (b s c) -> p b s c", b=TPB, c=DC),
                            in0=tp[:].rearrange("p (c b s) -> p b s c",
                                                c=DC, b=TPB),
                            scalar1=0.0,
                        )

            # w1 lands during the tail (queued behind half the stream)
            nc.sync.dma_start(w1t[:], w1p_in.ap())

            # ---- block-sum scan: C[k] = sum of bsums 0..k ------------
            slot0 = table[:, 0:NSLOT * DC].rearrange(
                "p (k r) -> p k r", k=NB)
            crgn = table[:, CBASE * DC:(CBASE + NB) * DC].rearrange(
                "p (k c) -> p k c", c=DC)
            for c in range(DC):
                nc.vector.tensor_tensor_scan(
                    out=crgn[:, :, c],
                    data0=slot0[:, :, c],
                    data1=u_v[:, 0:1].broadcast_to([P, NB]),
                    initial=0.0,
                    op0=add,
                    op1=bypass,
                )

            # ---- gather Ps | Cs | Pe | Ce, combine to segment sums ----
            nc.gpsimd.ap_gather(
                gath[:], table[:], gidx_v,
                channels=P, num_elems=NELEM, d=DC, num_idxs=4 * M,
            )
            q = M * DC
            nc.vector.tensor_tensor(
                out=tmpe[:], in0=gath[:, 2 * q:3 * q], in1=gath[:, 3 * q:4 * q],
                op=add)
            nc.vector.tensor_tensor(
                out=tmps[:], in0=gath[:, 0:q], in1=gath[:, q:2 * q],
                op=add)
            # deinterleave to chunk-major so proj lhsT slices are contiguous
            nc.vector.tensor_tensor(
                out=seg[:].rearrange("p (c m) -> p m c", c=DC),
                in0=tmpe[:].rearrange("p (m c) -> p m c", c=DC),
                in1=tmps[:].rearrange("p (m c) -> p m c", c=DC),
                op=sub,
            )

            # ---- projection + recip scale + bias ----------------------
            for mt in range(MC):
                po = psum.tile([P, D], F32, name=f"po_{mt}", tag="ps")
                for c in range(DC):
                    nc.tensor.matmul(
                        po[:],
                        lhsT=seg[:, c * M + mt * P:c * M + (mt + 1) * P],
                        rhs=w1t[:, c * D:(c + 1) * D],
                        start=(c == 0), stop=(c == DC - 1),
                    )
                ot = outp.tile([P, D], F32, name="ot", tag="ot")
                nc.vector.scalar_tensor_tensor(
                    out=ot[:],
                    in0=po[:],
                    scalar=recip_v[:, mt:mt + 1],
                    in1=biassb[:, mt * D:(mt + 1) * D],
                    op0=mult,
                    op1=add,
                )
                ring = nc.sync if mt % 2 == 0 else nc.scalar
                ring.dma_start(out.ap()[mt * P:(mt + 1) * P, :], ot[:])

    nc.compile()
    return nc


def _fourier_features(pos, dim):
    half = dim // 2
    freqs = np.exp(np.linspace(0.0, math.log(1000.0), half))
    ang = pos[..., None] * freqs
    return np.concatenate([np.sin(ang), np.cos(ang)], axis=-1)


def _wrap_idx(idx):
    """gpsimd index layout: idx j lives at [16g + j%16, j//16] for each of
    the 8 gpsimd cores g."""
    n = idx.shape[0]
    wrapped = idx.reshape(n // 16, 16).T
    return np.tile(wrapped, (8, 1)).astype(np.int16)


def _host_prep(frame_emb, beat_bounds, W, b, S):
    L = _layout(S)
    CBASE, ZCOL = L["cbase"], L["zcol"]

    s_all = np.clip(beat_bounds[:, :, 0], 0, T - 1).astype(np.int64)
    e_all = np.maximum(
        s_all + 1, np.minimum(beat_bounds[:, :, 1], T)).astype(np.int64)
    recip_all = (1.0 / (e_all - s_all)).astype(np.float32)

    pos = np.clip(np.arange(M, dtype=np.float64) / max(1, M - 1), 0.0, 1.0)
    ff = _fourier_features(pos, POS_DIM)                  # [M, 32]
    # rows 0:34 = [ff^T; ones; zero] (cols = m), rows 34:68 = [W2; b; zero]
    # (cols = j); bias[m, j] = sum_r ffw2[r, m] * ffw2[34 + r, j]
    ffw2 = np.zeros((P, D), dtype=np.float32)
    ffw2[0:POS_DIM, :] = ff.T.astype(np.float32)
    ffw2[POS_DIM, :] = 1.0
    ffw2[64:64 + POS_DIM, :] = W[D:D + POS_DIM, :].astype(np.float32)
    ffw2[64 + POS_DIM, :] = b.astype(np.float32)

    w1p = np.ascontiguousarray(
        W[:D, :].astype(np.float32).reshape(DC, P, D)
        .transpose(1, 0, 2).reshape(P, DC * D))

    # U[p, o] = 1.0 if p <= o else 0.0 (inclusive prefix-mask columns)
    U = (np.arange(P)[:, None] <= np.arange(P)[None, :]).astype(np.float32)
    ident = np.eye(P, dtype=np.float32)

    in_maps = []
    for i in range(B):
        s, e = s_all[i], e_all[i]
        allpos = np.concatenate([(s - 1)[s > 0], e - 1])
        uidx = np.zeros(NB * S, dtype=np.int64)
        slotmap = {}
        for k in range(NB):
            offs = np.unique(allpos[(allpos >> 7) == k] & 127)
            if len(offs) > S - 1:
                raise OverflowError(
                    f"block {k}: {len(offs)} boundaries > {S - 1}")
            uidx[k * S] = 127                              # block-sum slot
            for j, o in enumerate(offs):
                uidx[k * S + 1 + j] = o
                slotmap[(k, int(o))] = k * S + 1 + j

        def pcol(p):
            return slotmap[(int(p) >> 7, int(p) & 127)]

        def ccol(p):
            K = int(p) >> 7
            return ZCOL if K == 0 else CBASE + K - 1

        gidx = np.empty(4 * M, dtype=np.int64)
        for m in range(M):
            if s[m] == 0:
                gidx[m] = ZCOL
                gidx[M + m] = ZCOL
            else:
                gidx[m] = pcol(s[m] - 1)
                gidx[M + m] = ccol(s[m] - 1)
            gidx[2 * M + m] = pcol(e[m] - 1)
            gidx[3 * M + m] = ccol(e[m] - 1)

        recip_t = recip_all[i].reshape(MC, P).T.copy()     # [P, MC]

        bund = np.empty((P, L["cols"]), dtype=np.int16)
        bund[:, L["uidx0"]:L["gidx0"]] = _wrap_idx(uidx)
        bund[:, L["gidx0"]:L["recip0"]] = _wrap_idx(gidx)
        bund[:, L["recip0"]:L["u0"]] = recip_t.view(np.int16)
        bund[:, L["u0"]:L["ident0"]] = U.view(np.int16)
        bund[:, L["ident0"]:L["cols"]] = ident.view(np.int16)

        in_maps.append({
            "frame": np.ascontiguousarray(frame_emb[i], dtype=np.float32),
            "w1p": w1p,
            "ffw2": ffw2,
            "bund": bund,
        })
    return in_maps


def get_nc(S=32):
    if S not in _CACHED_NC:
        _CACHED_NC[S] = _build_nc(S)
    return _CACHED_NC[S]


def kernel(frame_emb, beat_bounds, W, b, _trace=False):
    frame_emb = np.asarray(frame_emb)
    beat_bounds = np.asarray(beat_bounds)
    W = np.asarray(W)
    b = np.asarray(b)
    in_maps = None
    for S in (32, 64):
        try:
            in_maps = _host_prep(frame_emb, beat_bounds, W, b, S)
            break
        except OverflowError:
            continue
    if in_maps is None:
        raise RuntimeError("too many segment boundaries per 128-frame block")
    nc = get_nc(S)
    res = bass_utils.run_bass_kernel_spmd(
        nc, in_maps, core_ids=list(range(N_CORES)), trace=_trace)
    out = np.stack([res.results[i]["out"] for i in range(B)], axis=0)
    if _trace:
        kernel.last_results = res
    return out


# revision 21
# speedup vs baseline: 2.0362x; 2.0362x over previous
"""BeatPooling segment-mean kernel for 8 Trainium2 NeuronCores.

Reference computation (per batch row):
    s = clip(bounds[:, 0], 0, T-1); e = max(s+1, min(bounds[:, 1], T))
    mean[m] = sum(frame[s_m:e_m]) / (e_m - s_m)
    out = concat([mean, fourier(pos)], -1) @ W + b         # [M, D]

Sharding: data-parallel over B (one batch row per core).

Algorithm (per core), all matmuls, no gpsimd (ap_gather costs ~30 ns per
index on the Q7 cores -- ~31 us per 1024 indices -- so every
gather-based formulation loses):

  1. Edge matmuls.  For each 128-frame block k, one f32r matmul with a
     host-built stationary operand uslots_k [128, 32]: column 0 is
     all-ones (the block sum), columns 1.. are inclusive prefix masks,
     one per distinct segment-boundary position (s-1 or e-1) falling in
     that block.  The moving operand is the frame tile [128, 512].  The
     PSUM result P'_k[slot, d] holds every within-block prefix the
     output needs.  f32r streams at 1 cycle/row, so the whole 16 MiB
     frame row costs ~64 x 0.3 us of PE time and is never transposed,
     scanned, or cast.
  2. P' tiles are evacuated to SBUF as fp16 (0.05% worst-case error --
     well within the 2e-2 gate).
  3. Combine matmuls.  segT[d, m] = sum_t pv_t^T . G_t, accumulated in
     PSUM over the 16 slot-tiles as they appear.  G_t [128 slots, 512 m]
     (host-built fp16, +-1 one-hots at each segment's e/s boundary
     slots) also absorbs the block-span part: its slot-0 rows carry the
     0/1 band J[k, m] = [K_s(m) <= k < K_e(m)], which multiplies the
     block sums.  So segT accumulates (P_e - P_s + sum of spanned block
     sums) == the full segment sums, transposed, ready for projection.
  4. Projection in fp16 (W1 host-packed), then one scalar_tensor_tensor
     fuses the 1/count scale (per-partition scalar) and the fourier/bias
     term (computed on device from a tiny packed tensor by one more
     matmul per m-tile).

DMA notes: all aux tensors ride in a few large contiguous DMAs (the
original baseline lost ~50 us draining dozens of tiny per-partition
descriptors), and the 16 MiB frame stream alternates between the two
HWDGE rings (sync/scalar) in 2 MiB chunks.
"""

import math

import numpy as np

import concourse.bacc as bacc
import concourse.mybir as mybir
from concourse import bass_utils
from concourse.tile import TileContext

B, T, D, M = 8, 8192, 512, 512
POS_DIM = 32
P = 128
N_CORES = 8
NB = T // P            # 64 blocks of 128 frames
GROUPS = 8             # stream groups (8 blocks = 2 MiB each)
BPG = NB // GROUPS     # blocks per group
DC = D // P            # 4 d-chunks
MC = M // P            # 4 m-chunks

F32 = mybir.dt.float32
F32R = mybir.dt.float32r
F16 = mybir.dt.float16

_CACHED_NC = {}


def _build_nc(S):
    NSLOT = NB * S
    TPB = P // S           # blocks per slot-tile (4 for S=32)
    NT = NB // TPB         # slot-tiles (16)

    nc = bacc.Bacc("TRN2", target_bir_lowering=False, debug=False,
                   num_devices=N_CORES)

    frame = nc.dram_tensor("frame", [T, D], F32R, kind="ExternalInput")
    us_in = nc.dram_tensor("uslots", [P, NSLOT + 4], F32R,
                           kind="ExternalInput")
    g_in = nc.dram_tensor("gmat", [P, NT * M], F16, kind="ExternalInput")
    w1_in = nc.dram_tensor("w1p", [P, DC * D], F16, kind="ExternalInput")
    ffw2_in = nc.dram_tensor("ffw2", [P, D], F32R, kind="ExternalInput")
    out = nc.dram_tensor("out", [M, D], F32, kind="ExternalOutput")

    add = mybir.AluOpType.add
    mult = mybir.AluOpType.mult

    with TileContext(nc, num_cores=N_CORES) as tc:
        with (
            tc.tile_pool(name="const", bufs=1) as const,
            tc.tile_pool(name="staging", bufs=2) as staging,
            tc.tile_pool(name="psum", bufs=4, space="PSUM") as psum,
            tc.tile_pool(name="pacc", bufs=1, space="PSUM") as pacc,
            tc.tile_pool(name="outp", bufs=2) as outp,
        ):
            # ---- long-lived tiles -------------------------------------
            uslots = const.tile([P, NSLOT + 4], F32R, name="uslots")
            gmat = const.tile([P, NT * M], F16, name="gmat")
            pvall = const.tile([P, NT * D], F16, name="pvall")
            w1t = const.tile([P, DC * D], F16, name="w1t")
            ffa = const.tile([64, D], F32R, name="ffa")
            ffb = const.tile([64, D], F32R, name="ffb")
            segsb = const.tile([P, DC * M], F16, name="segsb")
            biassb = const.tile([P, MC * D], F32, name="biassb")

            recip_v = uslots[:, NSLOT:NSLOT + 4].bitcast(F32)

            # ---- constant DMAs ---------------------------------------
            nc.sync.dma_start(uslots[:], us_in.ap())
            nc.scalar.dma_start(ffa[:], ffw2_in.ap()[0:64, :])
            nc.scalar.dma_start(ffb[:], ffw2_in.ap()[64:128, :])
            nc.scalar.dma_start(gmat[:], g_in.ap())
            nc.scalar.dma_start(w1t[:], w1_in.ap())

            # fourier/bias term: bias[m, j] = ff[m] @ W2 + b
            for mt in range(MC):
                bps = psum.tile([P, D], F32, name=f"bps_{mt}", tag="ps")
                nc.tensor.matmul(
                    bps[:],
                    lhsT=ffa[:, mt * P:(mt + 1) * P],
                    rhs=ffb[:],
                    start=True, stop=True,
                )
                nc.scalar.copy(biassb[:, mt * D:(mt + 1) * D], bps[:])

            # segT accumulators, one per d-chunk, live across the stream
            po = [pacc.tile([P, M], F32, name=f"po_{c}", tag=f"po{c}")
                  for c in range(DC)]

            # ---- stream frame ----------------------------------------
            frame_g = frame.ap().rearrange("(g b p) d -> g p b d", p=P, b=BPG)
            for g in range(GROUPS):
                st = staging.tile([P, BPG * D], F32R, name="stage",
                                  tag="stage")
                ring = nc.sync if g % 2 == 0 else nc.scalar
                ring.dma_start(
                    st[:].rearrange("p (b d) -> p b d", b=BPG), frame_g[g])
                for b in range(BPG):
                    k = g * BPG + b
                    i = k % TPB
                    t = k // TPB
                    pp = psum.tile([S, D], F32, name=f"pp_{k}", tag="ps")
                    nc.tensor.matmul(
                        pp[:],
                        lhsT=uslots[:, k * S:(k + 1) * S],
                        rhs=st[:, b * D:(b + 1) * D],
                        start=True, stop=True,
                    )
                    nc.scalar.copy(
                        pvall[i * S:(i + 1) * S, t * D:(t + 1) * D], pp[:])
                    if i == TPB - 1:
                        # combine: segT[c] += pv_t[:, c]^T @ G_t
                        for c in range(DC):
                            nc.tensor.matmul(
                                po[c][:],
                                lhsT=pvall[:, t * D + c * P:
                                           t * D + (c + 1) * P],
                                rhs=gmat[:, t * M:(t + 1) * M],
                                start=(t == 0), stop=(t == NT - 1),
                                skip_group_check=True,
                            )

            # ---- segT -> SBUF fp16, project, scale, bias --------------
            for c in range(DC):
                nc.vector.tensor_scalar_add(
                    out=segsb[:, c * M:(c + 1) * M], in0=po[c][:],
                    scalar1=0.0)
            for mt in range(MC):
                po2 = psum.tile([P, D], F32, name=f"po2_{mt}", tag="ps")
                for c in range(DC):
                    nc.tensor.matmul(
                        po2[:],
                        lhsT=segsb[:, c * M + mt * P:c * M + (mt + 1) * P],
                        rhs=w1t[:, c * D:(c + 1) * D],
                        start=(c == 0), stop=(c == DC - 1),
                    )
                ot = outp.tile([P, D], F32, name="ot", tag="ot")
                nc.vector.scalar_tensor_tensor(
                    out=ot[:],
                    in0=po2[:],
                    scalar=recip_v[:, mt:mt + 1],
                    in1=biassb[:, mt * D:(mt + 1) * D],
                    op0=mult,
                    op1=add,
                )
                ring = nc.sync if mt % 2 == 0 else nc.scalar
                ring.dma_start(out.ap()[mt * P:(mt + 1) * P, :], ot[:])

    nc.compile()
    return nc


def _fourier_features(pos, dim):
    half = dim // 2
    freqs = np.exp(np.linspace(0.0, math.log(1000.0), half))
    ang = pos[..., None] * freqs
    return np.concatenate([np.sin(ang), np.cos(ang)], axis=-1)


def _host_prep(frame_emb, beat_bounds, W, b, S):
    NSLOT = NB * S
    TPB = P // S
    NT = NB // TPB

    s_all = np.clip(beat_bounds[:, :, 0], 0, T - 1).astype(np.int64)
    e_all = np.maximum(
        s_all + 1, np.minimum(beat_bounds[:, :, 1], T)).astype(np.int64)
    recip_all = (1.0 / (e_all - s_all)).astype(np.float32)

    pos = np.clip(np.arange(M, dtype=np.float64) / max(1, M - 1), 0.0, 1.0)
    ff = _fourier_features(pos, POS_DIM)                  # [M, 32]
    # rows 0:64 = [ff^T; ones; pad] (cols = m), rows 64:128 = [W2; b; pad]
    ffw2 = np.zeros((P, D), dtype=np.float32)
    ffw2[0:POS_DIM, :] = ff.T.astype(np.float32)
    ffw2[POS_DIM, :] = 1.0
    ffw2[64:64 + POS_DIM, :] = W[D:D + POS_DIM, :].astype(np.float32)
    ffw2[64 + POS_DIM, :] = b.astype(np.float32)

    w1p = np.ascontiguousarray(
        W[:D, :].astype(np.float16).reshape(DC, P, D)
        .transpose(1, 0, 2).reshape(P, DC * D))

    # U[p, o] = 1.0 if p <= o (inclusive prefix-mask columns)
    U = (np.arange(P)[:, None] <= np.arange(P)[None, :]).astype(np.float32)

    in_maps = []
    for i in range(B):
        s, e = s_all[i], e_all[i]
        allpos = np.concatenate([(s - 1)[s > 0], e - 1])
        uslots = np.zeros((P, NSLOT + 4), dtype=np.float32)
        slotmap = {}
        for k in range(NB):
            offs = np.unique(allpos[(allpos >> 7) == k] & 127)
            if len(offs) > S - 1:
                raise OverflowError(
                    f"block {k}: {len(offs)} boundaries > {S - 1}")
            uslots[:, k * S] = 1.0                         # block-sum slot
            for j, o in enumerate(offs):
                uslots[:, k * S + 1 + j] = U[:, o]
                slotmap[(k, int(o))] = k * S + 1 + j
        uslots[:, NSLOT:NSLOT + 4] = recip_all[i].reshape(MC, P).T

        # G_t[slot, m]: +1 at e-boundary slot, -1 at s-boundary slot,
        # 0/1 block-span band J on the slot-0 rows
        gm = np.zeros((NT, P, M), dtype=np.float32)
        for m in range(M):
            pe = int(e[m]) - 1
            ke = pe >> 7
            sl = slotmap[(ke, pe & 127)]
            gm[sl // P, sl % P, m] += 1.0
            ks = 0
            if s[m] > 0:
                ps = int(s[m]) - 1
                ks = ps >> 7
                sl = slotmap[(ks, ps & 127)]
                gm[sl // P, sl % P, m] -= 1.0
            for k in range(ks, ke):
                sl = k * S
                gm[sl // P, sl % P, m] += 1.0
        gmat = np.ascontiguousarray(
            gm.transpose(1, 0, 2).reshape(P, NT * M)).astype(np.float16)

        in_maps.append({
            "frame": np.ascontiguousarray(frame_emb[i], dtype=np.float32),
            "uslots": uslots,
            "gmat": gmat,
            "w1p": w1p,
            "ffw2": ffw2,
        })
    return in_maps


def get_nc(S=32):
    if S not in _CACHED_NC:
        _CACHED_NC[S] = _build_nc(S)
    return _CACHED_NC[S]


def kernel(frame_emb, beat_bounds, W, b, _trace=False):
    frame_emb = np.asarray(frame_emb)
    beat_bounds = np.asarray(beat_bounds)
    W = np.asarray(W)
    b = np.asarray(b)
    in_maps = None
    for S in (32, 64):
        try:
            in_maps = _host_prep(frame_emb, beat_bounds, W, b, S)
            break
        except OverflowError:
            continue
    if in_maps is None:
        raise RuntimeError("too many segment boundaries per 128-frame block")
    nc = get_nc(S)
    res = bass_utils.run_bass_kernel_spmd(
        nc, in_maps, core_ids=list(range(N_CORES)), trace=_trace)
    out = np.stack([res.results[i]["out"] for i in range(B)], axis=0)
    if _trace:
        kernel.last_results = res
    return out


# revision 23
# speedup vs baseline: 3.0294x; 1.4878x over previous
"""BeatPooling segment-mean kernel for 8 Trainium2 NeuronCores.

Reference computation (per batch row):
    s = clip(bounds[:, 0], 0, T-1); e = max(s+1, min(bounds[:, 1], T))
    mean[m] = sum(frame[s_m:e_m]) / (e_m - s_m)
    out = concat([mean, fourier(pos)], -1) @ W + b         # [M, D]

Sharding: data-parallel over B (one batch row per core).

Algorithm (per core), all matmuls, no gpsimd (ap_gather costs ~30 ns per
index on the Q7 cores -- ~31 us per 1024 indices -- so every
gather-based formulation loses):

  1. Edge matmuls.  For each 128-frame block k, one f32r matmul with a
     host-built stationary operand uslots_k [128, 32]: column 0 is
     all-ones (the block sum), columns 1.. are inclusive prefix masks,
     one per distinct segment-boundary position (s-1 or e-1) falling in
     that block.  The moving operand is the frame tile [128, 512].  The
     PSUM result P'_k[slot, d] holds every within-block prefix the
     output needs.  f32r streams at 1 cycle/row, so the whole 16 MiB
     frame row costs ~64 x 0.3 us of PE time and is never transposed,
     scanned, or cast.
  2. P' tiles are evacuated to SBUF as fp16 (0.05% worst-case error --
     well within the 2e-2 gate).
  3. Combine matmuls.  segT[d, m] = sum_t pv_t^T . G_t, accumulated in
     PSUM over the 16 slot-tiles as they appear.  G_t [128 slots, 512 m]
     (host-built fp16, +-1 one-hots at each segment's e/s boundary
     slots) also absorbs the block-span part: its slot-0 rows carry the
     0/1 band J[k, m] = [K_s(m) <= k < K_e(m)], which multiplies the
     block sums.  So segT accumulates (P_e - P_s + sum of spanned block
     sums) == the full segment sums, transposed, ready for projection.
  4. Projection in fp16 (W1 host-packed), then one scalar_tensor_tensor
     fuses the 1/count scale (per-partition scalar) and the fourier/bias
     term (computed on device from a tiny packed tensor by one more
     matmul per m-tile).

DMA notes: all aux tensors ride in a few large contiguous DMAs (the
original baseline lost ~50 us draining dozens of tiny per-partition
descriptors), and the 16 MiB frame stream alternates between the two
HWDGE rings (sync/scalar) in 2 MiB chunks.
"""

import math

import numpy as np

import concourse.bacc as bacc
import concourse.mybir as mybir
from concourse import bass_utils
from concourse.tile import TileContext

B, T, D, M = 8, 8192, 512, 512
POS_DIM = 32
P = 128
N_CORES = 8
NB = T // P            # 64 blocks of 128 frames
GROUPS = 16            # stream groups (4 blocks = 1 MiB each)
BPG = NB // GROUPS     # blocks per group
DC = D // P            # 4 d-chunks
MC = M // P            # 4 m-chunks

F32 = mybir.dt.float32
F32R = mybir.dt.float32r
F16 = mybir.dt.float16

_CACHED_NC = {}


def _build_nc(S):
    NSLOT = NB * S
    TPB = P // S           # blocks per slot-tile (4 for S=32)
    NT = NB // TPB         # slot-tiles (16)

    nc = bacc.Bacc("TRN2", target_bir_lowering=False, debug=False,
                   num_devices=N_CORES)

    frame = nc.dram_tensor("frame", [T, D], F32R, kind="ExternalInput")
    us_in = nc.dram_tensor("uslots", [P, NSLOT + 4], F32R,
                           kind="ExternalInput")
    g_in = nc.dram_tensor("gmat", [P, NT * M], F16, kind="ExternalInput")
    w1_in = nc.dram_tensor("w1p", [P, DC * D], F16, kind="ExternalInput")
    ffw2_in = nc.dram_tensor("ffw2", [P, D], F32R, kind="ExternalInput")
    out = nc.dram_tensor("out", [M, D], F32, kind="ExternalOutput")

    add = mybir.AluOpType.add
    mult = mybir.AluOpType.mult

    with TileContext(nc, num_cores=N_CORES) as tc:
        with (
            tc.tile_pool(name="const", bufs=1) as const,
            tc.tile_pool(name="staging", bufs=4) as staging,
            tc.tile_pool(name="psum", bufs=4, space="PSUM") as psum,
            tc.tile_pool(name="pacc", bufs=1, space="PSUM") as pacc,
            tc.tile_pool(name="outp", bufs=2) as outp,
        ):
            # ---- long-lived tiles -------------------------------------
            uslots = const.tile([P, NSLOT + 4], F32R, name="uslots")
            gmat = const.tile([P, NT * M], F16, name="gmat")
            pvall = const.tile([P, NT * D], F16, name="pvall")
            w1t = const.tile([P, DC * D], F16, name="w1t")
            ffa = const.tile([64, D], F32R, name="ffa")
            ffb = const.tile([64, D], F32R, name="ffb")
            segsb = const.tile([P, DC * M], F16, name="segsb")
            biassb = const.tile([P, MC * D], F32, name="biassb")

            recip_v = uslots[:, NSLOT:NSLOT + 4].bitcast(F32)

            # ---- constant DMAs (uslots gates the first edge matmul;
            # gmat/w1 are needed only later and ride mid/late on the
            # sync ring to balance ring bytes) ----
            nc.scalar.dma_start(uslots[:], us_in.ap())
            nc.scalar.dma_start(ffa[:], ffw2_in.ap()[0:64, :])
            nc.scalar.dma_start(ffb[:], ffw2_in.ap()[64:128, :])

            # fourier/bias term: bias[m, j] = ff[m] @ W2 + b
            for mt in range(MC):
                bps = psum.tile([P, D], F32, name=f"bps_{mt}", tag="ps")
                nc.tensor.matmul(
                    bps[:],
                    lhsT=ffa[:, mt * P:(mt + 1) * P],
                    rhs=ffb[:],
                    start=True, stop=True,
                )
                nc.scalar.copy(biassb[:, mt * D:(mt + 1) * D], bps[:])

            # segT accumulators, one per d-chunk, live across the stream
            po = [pacc.tile([P, M], F32, name=f"po_{c}", tag=f"po{c}")
                  for c in range(DC)]

            # gmat arrives in just-in-time quarters on the sync ring so
            # the combine matmuls never wait and the stream is not delayed
            QW = NT * M // 4
            nc.sync.dma_start(gmat[:, 0:QW], g_in.ap()[:, 0:QW])

            # ---- stream frame ----------------------------------------
            frame_g = frame.ap().rearrange("(g b p) d -> g p b d", p=P, b=BPG)
            for g in range(GROUPS):
                st = staging.tile([P, BPG * D], F32R, name="stage",
                                  tag="stage")
                if g in (4, 8, 12):
                    q = g // 4
                    nc.sync.dma_start(gmat[:, q * QW:(q + 1) * QW],
                                      g_in.ap()[:, q * QW:(q + 1) * QW])
                if g == 14:
                    nc.sync.dma_start(w1t[:], w1_in.ap())
                ring = nc.sync if g % 2 == 0 else nc.scalar
                ring.dma_start(
                    st[:].rearrange("p (b d) -> p b d", b=BPG), frame_g[g])
                for b in range(BPG):
                    k = g * BPG + b
                    i = k % TPB
                    t = k // TPB
                    pp = psum.tile([S, D], F32, name=f"pp_{k}", tag="ps")
                    nc.tensor.matmul(
                        pp[:],
                        lhsT=uslots[:, k * S:(k + 1) * S],
                        rhs=st[:, b * D:(b + 1) * D],
                        start=True, stop=True,
                    )
                    nc.vector.tensor_scalar_add(
                        out=pvall[i * S:(i + 1) * S, t * D:(t + 1) * D],
                        in0=pp[:], scalar1=0.0)
                    if i == TPB - 1:
                        # combine: segT[c] += pv_t[:, c]^T @ G_t
                        for c in range(DC):
                            nc.tensor.matmul(
                                po[c][:],
                                lhsT=pvall[:, t * D + c * P:
                                           t * D + (c + 1) * P],
                                rhs=gmat[:, t * M:(t + 1) * M],
                                start=(t == 0), stop=(t == NT - 1),
                                skip_group_check=True,
                            )

            # ---- segT -> SBUF fp16, project, scale, bias --------------
            for c in range(DC):
                nc.vector.tensor_scalar_add(
                    out=segsb[:, c * M:(c + 1) * M], in0=po[c][:],
                    scalar1=0.0)
            for mt in range(MC):
                po2 = psum.tile([P, D], F32, name=f"po2_{mt}", tag="ps")
                for c in range(DC):
                    nc.tensor.matmul(
                        po2[:],
                        lhsT=segsb[:, c * M + mt * P:c * M + (mt + 1) * P],
                        rhs=w1t[:, c * D:(c + 1) * D],
                        start=(c == 0), stop=(c == DC - 1),
                    )
                ot = outp.tile([P, D], F32, name="ot", tag="ot")
                nc.vector.scalar_tensor_tensor(
                    out=ot[:],
                    in0=po2[:],
                    scalar=recip_v[:, mt:mt + 1],
                    in1=biassb[:, mt * D:(mt + 1) * D],
                    op0=mult,
                    op1=add,
                )
                ring = nc.sync if mt % 2 == 0 else nc.scalar
                ring.dma_start(out.ap()[mt * P:(mt + 1) * P, :], ot[:])

    nc.compile()
    return nc


def _fourier_features(pos, dim):
    half = dim // 2
    freqs = np.exp(np.linspace(0.0, math.log(1000.0), half))
    ang = pos[..., None] * freqs
    return np.concatenate([np.sin(ang), np.cos(ang)], axis=-1)


def _host_prep(frame_emb, beat_bounds, W, b, S):
    NSLOT = NB * S
    TPB = P // S
    NT = NB // TPB

    s_all = np.clip(beat_bounds[:, :, 0], 0, T - 1).astype(np.int64)
    e_all = np.maximum(
        s_all + 1, np.minimum(beat_bounds[:, :, 1], T)).astype(np.int64)
    recip_all = (1.0 / (e_all - s_all)).astype(np.float32)

    pos = np.clip(np.arange(M, dtype=np.float64) / max(1, M - 1), 0.0, 1.0)
    ff = _fourier_features(pos, POS_DIM)                  # [M, 32]
    # rows 0:64 = [ff^T; ones; pad] (cols = m), rows 64:128 = [W2; b; pad]
    ffw2 = np.zeros((P, D), dtype=np.float32)
    ffw2[0:POS_DIM, :] = ff.T.astype(np.float32)
    ffw2[POS_DIM, :] = 1.0
    ffw2[64:64 + POS_DIM, :] = W[D:D + POS_DIM, :].astype(np.float32)
    ffw2[64 + POS_DIM, :] = b.astype(np.float32)

    w1p = np.ascontiguousarray(
        W[:D, :].astype(np.float16).reshape(DC, P, D)
        .transpose(1, 0, 2).reshape(P, DC * D))

    # U[p, o] = 1.0 if p <= o (inclusive prefix-mask columns)
    U = (np.arange(P)[:, None] <= np.arange(P)[None, :]).astype(np.float32)

    in_maps = []
    for i in range(B):
        s, e = s_all[i], e_all[i]
        allpos = np.concatenate([(s - 1)[s > 0], e - 1])
        uslots = np.zeros((P, NSLOT + 4), dtype=np.float32)
        slotmap = {}
        for k in range(NB):
            offs = np.unique(allpos[(allpos >> 7) == k] & 127)
            if len(offs) > S - 1:
                raise OverflowError(
                    f"block {k}: {len(offs)} boundaries > {S - 1}")
            uslots[:, k * S] = 1.0                         # block-sum slot
            for j, o in enumerate(offs):
                uslots[:, k * S + 1 + j] = U[:, o]
                slotmap[(k, int(o))] = k * S + 1 + j
        uslots[:, NSLOT:NSLOT + 4] = recip_all[i].reshape(MC, P).T

        # G_t[slot, m]: +1 at e-boundary slot, -1 at s-boundary slot,
        # 0/1 block-span band J on the slot-0 rows
        gm = np.zeros((NT, P, M), dtype=np.float32)
        for m in range(M):
            pe = int(e[m]) - 1
            ke = pe >> 7
            sl = slotmap[(ke, pe & 127)]
            gm[sl // P, sl % P, m] += 1.0
            ks = 0
            if s[m] > 0:
                ps = int(s[m]) - 1
                ks = ps >> 7
                sl = slotmap[(ks, ps & 127)]
                gm[sl // P, sl % P, m] -= 1.0
            for k in range(ks, ke):
                sl = k * S
                gm[sl // P, sl % P, m] += 1.0
        gmat = np.ascontiguousarray(
            gm.transpose(1, 0, 2).reshape(P, NT * M)).astype(np.float16)

        in_maps.append({
            "frame": np.ascontiguousarray(frame_emb[i], dtype=np.float32),
            "uslots": uslots,
            "gmat": gmat,
            "w1p": w1p,
            "ffw2": ffw2,
        })
    return in_maps


def get_nc(S=32):
    if S not in _CACHED_NC:
        _CACHED_NC[S] = _build_nc(S)
    return _CACHED_NC[S]


def kernel(frame_emb, beat_bounds, W, b, _trace=False):
    frame_emb = np.asarray(frame_emb)
    beat_bounds = np.asarray(beat_bounds)
    W = np.asarray(W)
    b = np.asarray(b)
    in_maps = None
    for S in (32, 64):
        try:
            in_maps = _host_prep(frame_emb, beat_bounds, W, b, S)
            break
        except OverflowError:
            continue
    if in_maps is None:
        raise RuntimeError("too many segment boundaries per 128-frame block")
    nc = get_nc(S)
    res = bass_utils.run_bass_kernel_spmd(
        nc, in_maps, core_ids=list(range(N_CORES)), trace=_trace)
    out = np.stack([res.results[i]["out"] for i in range(B)], axis=0)
    if _trace:
        kernel.last_results = res
    return out
